# revision 1
# baseline (speedup 1.0000x reference)
"""Trainium2 Bass kernel for nn_ByteGridModel (dense_cnn).

Sharding: pure data-parallel over batch B=8 -> 8 cores, one batch item per
core, no collectives. Weights replicated (streamed per layer, double
buffered).

Per-core layout: channels on partitions, h = [H=512 -> 4x128, S=256] fp32r
resident in SBUF as one [128, 4, 256] tile (fp32r so h can be a full-speed
matmul moving operand).

Per layer:
  - rmsnorm: squares split across ACT/DVE/Pool -> fp32r ones-matmul
    partition reduction -> sqrt -> DVE reciprocal -> fp32r broadcast
    matmul. rms weights / alphas are folded into mixer/GLU weights on host.
  - per-channel 16x16 mixers: DVE broadcast-AP products with the reduction
    axis packed innermost (both operands stride-1 -> DVE 2x mode), in two
    half-products per tile so PE identity-matmul accumulation starts early.
    For the global mixer the normalized activations are written
    within-chunk transposed (v_t[c, 16j+i]) so its product is 2x too.
  - GLU MLP: Wv/Wg matmuls run on RAW h (rstd factored out of the channel
    sum) so they overlap the rms chain; rstd applied to p1 before Silu and
    once more to the Wo output (h += (Wo @ [silu(p1*rstd) . p3]) * rstd).
  - act-table thrash (sqrt vs silu in different ACT tables) hidden by
    issuing dummy [1,1] activations early so table loads overlap compute.
"""

import numpy as np
import ml_dtypes

import concourse.bacc as bacc
import concourse.bass as bass
import concourse.tile as tile
import concourse.mybir as mybir
from concourse.bass_utils import run_bass_kernel_spmd

B, S, H, GLU, VOC, L, CIN, BLK = 8, 256, 512, 1024, 256, 24, 320, 16
EPS = 1e-5
NT = H // 128  # 4 channel tiles
GT = GLU // 128  # 8 glu tiles

F32 = mybir.dt.float32
F32R = mybir.dt.float32r
BF16 = mybir.dt.bfloat16
MULT = mybir.AluOpType.mult
ADD = mybir.AluOpType.add
AF = mybir.ActivationFunctionType

_PROG_CACHE = {}
NOLD = True  # skip ldweights on repeated-stationary matmuls


def _bview(base, doff, free_dims):
    """View of a 2D sbuf AP with custom (possibly broadcast) free dims."""
    return bass.AP(
        tensor=base.tensor,
        offset=base.offset + doff,
        ap=[list(base.ap[0])] + [list(d) for d in free_dims],
    )


def build_program(n_layers=L, sim_compat=False):
    nc = bacc.Bacc("TRN2", enable_partition_id=False)

    x_d = nc.dram_tensor("x", [384, S], F32R, kind="ExternalInput")
    stw_d = nc.dram_tensor("stem_wT", [384, H], F32R, kind="ExternalInput")
    wv_d = nc.dram_tensor("wvT", [n_layers, H, GLU], BF16, kind="ExternalInput")
    wg_d = nc.dram_tensor("wgT", [n_layers, H, GLU], BF16, kind="ExternalInput")
    wo_d = nc.dram_tensor("woT", [n_layers, GLU, H], BF16, kind="ExternalInput")
    wl_d = nc.dram_tensor("wl", [n_layers, H, 256], BF16, kind="ExternalInput")
    wm_d = nc.dram_tensor("wm", [n_layers, H, 256], BF16, kind="ExternalInput")
    hw_d = nc.dram_tensor("headT", [H, VOC], BF16, kind="ExternalInput")
    id_d = nc.dram_tensor("ident", [128, 128], BF16, kind="ExternalInput")
    idf_d = nc.dram_tensor("identf", [128, 128], F32R, kind="ExternalInput")
    ones_d = nc.dram_tensor("ones_k", [128, 1], BF16, kind="ExternalInput")
    onesr_d = nc.dram_tensor("ones_m", [1, 128], BF16, kind="ExternalInput")
    out_d = nc.dram_tensor("out", [VOC, S], F32, kind="ExternalOutput")

    silu_f = AF.Sigmoid if sim_compat else AF.Silu

    from contextlib import ExitStack

    with tile.TileContext(nc) as tc, ExitStack() as ctx:
        singles = ctx.enter_context(tc.tile_pool(name="singles", bufs=1))
        wpool = ctx.enter_context(tc.tile_pool(name="wpool", bufs=2))
        hpool = ctx.enter_context(tc.tile_pool(name="hpool", bufs=1))
        npool = ctx.enter_context(tc.tile_pool(name="npool", bufs=2))
        apool = ctx.enter_context(tc.tile_pool(name="apool", bufs=3))
        ppool = ctx.enter_context(tc.tile_pool(name="ppool", bufs=4))
        gpool = ctx.enter_context(tc.tile_pool(name="gpool", bufs=2))
        ps_n = ctx.enter_context(tc.tile_pool(name="ps_n", bufs=1, space="PSUM"))
        ps_m = ctx.enter_context(tc.tile_pool(name="ps_m", bufs=2, space="PSUM"))
        ps_g = ctx.enter_context(tc.tile_pool(name="ps_g", bufs=2, space="PSUM"))
        ps_o = ctx.enter_context(tc.tile_pool(name="ps_o", bufs=2, space="PSUM"))

        # ---- constants / stem operands ----
        ident = singles.tile([128, 128], BF16, tag="ident")
        nc.sync.dma_start(out=ident, in_=id_d[:])
        identf_st = singles.tile([128, 128], F32R, tag="identf_st")
        nc.sync.dma_start(out=identf_st, in_=idf_d[:])
        identf = singles.tile([128, 128], F32R, tag="identf")
        ones_k_st = singles.tile([128, 1], BF16, tag="ones_k_st")
        nc.sync.dma_start(out=ones_k_st, in_=ones_d[:])
        ones_k = singles.tile([128, 1], BF16, tag="ones_k")
        ones_m_st = singles.tile([1, 128], BF16, tag="ones_m_st")
        nc.sync.dma_start(out=ones_m_st, in_=onesr_d[:])
        ones_m = singles.tile([1, 128], BF16, tag="ones_m")
        eps_sb = singles.tile([1, 1], F32, tag="eps")
        nc.vector.memset(eps_sb, float(EPS))
        dum_a = singles.tile([1, 1], F32, tag="dum_a")
        nc.vector.memset(dum_a, 1.0)
        dum_b = singles.tile([1, 1], F32, tag="dum_b")

        x_st = singles.tile([128, 3, S], F32R, tag="x_st")
        nc.sync.dma_start(out=x_st, in_=x_d[:].rearrange("(t p) s -> p t s", p=128))
        x_sb = singles.tile([128, 3, S], F32R, tag="x")
        stw_st = singles.tile([128, 3, H], F32R, tag="stw_st")
        nc.sync.dma_start(out=stw_st, in_=stw_d[:].rearrange("(t p) s -> p t s", p=128))
        stw_sb = singles.tile([128, 3, H], F32R, tag="stw")

        # Route fp32r matmul operands through a DVE copy so each matmul's
        # operand has an engine writer (a matmul can carry only one
        # cross-engine wait through walrus codegen). Touch bf16 weight DMAs
        # with ldweights for the same reason.
        with nc.allow_low_precision(reason="fp32r staging copies"):
            nc.vector.tensor_copy(out=ones_k, in_=ones_k_st)
            nc.vector.tensor_copy(out=identf, in_=identf_st)
            nc.vector.tensor_copy(out=ones_m, in_=ones_m_st)
            nc.vector.tensor_copy(out=x_sb, in_=x_st)
            nc.vector.tensor_copy(out=stw_sb, in_=stw_st)
        nc.tensor.ldweights(ident[:, 0:128])

        # ---- h resident fp32r: [128, NT, S]; hb = bf16 shadow for use as
        # matmul moving operand (walrus rejects f32r x bf16 mixing) ----
        h = hpool.tile([128, NT, S], F32R, tag="h", name="h")
        hb = hpool.tile([128, NT, S], BF16, tag="hb", name="hb")

        # ---- stem: h = stem_w @ x ----
        for t in range(NT):
            pst = ps_o.tile([128, S], F32, tag="po")
            for kt in range(3):
                nc.tensor.matmul(
                    pst,
                    stw_sb[:, kt, t * 128 : (t + 1) * 128],
                    x_sb[:, kt, :],
                    start=(kt == 0),
                    stop=(kt == 2),
                )
            with nc.allow_low_precision(reason="h is fp32r"):
                nc.vector.tensor_copy(out=h[:, t, :], in_=pst)
        # warm the sqrt act table while stem finishes
        nc.scalar.activation(dum_b, dum_a, AF.Sqrt)

        def emit_rms(c0, w):
            """One rms chain for h columns [c0, c0+w): bf16 squares split
            across ACT(t0,t1)/DVE(t2)/Pool(t3), ones-matmul reduce, sqrt,
            reciprocal (bf16), per-partition-replicated rb via matmul.
            Returns the psum rb tile [128, w] fp32."""
            sqs = []
            with nc.allow_low_precision(reason="bf16 squares"):
                for t in range(NT):
                    sq = apool.tile([128, w], BF16, tag=f"sq{t}", name=f"sq{t}")
                    hv = h[:, t, c0 : c0 + w]
                    if t in (1, 3):
                        nc.gpsimd.tensor_tensor(out=sq, in0=hv, in1=hv, op=MULT)
                    else:
                        nc.scalar.square(sq, hv)
                    sqs.append(sq)
            ms = ps_n.tile([1, w], F32, tag="ms")
            for t in range(NT):
                nc.tensor.matmul(
                    ms, ones_k[:, 0:1], sqs[t][:],
                    start=(t == 0), stop=(t == NT - 1),
                )
            stdv = npool.tile([1, w], F32, tag="stdv")
            nc.scalar.activation(stdv, ms, AF.Sqrt, bias=eps_sb[0:1, 0:1], scale=1.0 / H)
            rstd = npool.tile([1, w], BF16, tag="rstd")
            with nc.allow_low_precision(reason="bf16 rstd for broadcast matmul"):
                nc.vector.reciprocal(rstd, stdv)
            rb = ps_n.tile([128, w], F32, tag="rb")
            nc.tensor.matmul(rb, ones_m[0:1, :], rstd[:], start=True, stop=True)
            # PSUM->SBUF so consumers never pair two PSUM operands
            rbs = npool.tile([128, w], F32, tag="rbs")
            nc.scalar.copy(rbs, rb)
            return rbs

        def emit_local_tile(t, hf, rb, u, wl_sb, cp_eng):
            """Local-mixer column-half: u-mult (Pool), 2x product (DVE),
            16 identity matmuls + h-inclusion matmul in one PSUM group,
            copy-back to h on cp_eng."""
            c0 = 128 * hf
            nc.vector.tensor_tensor(
                out=u[:, c0 : c0 + 128], in0=h[:, t, c0 : c0 + 128],
                in1=rb, op=MULT,
            )
            prod = ppool.tile([128, 8, 16, 16], BF16, tag="prod")
            uv = _bview(u[:], c0, [[16, 8], [0, 16], [1, 16]])
            wv_ = _bview(wl_sb[:, t, :], 0, [[0, 8], [16, 16], [1, 16]])
            nc.vector.tensor_tensor(out=prod, in0=uv, in1=wv_, op=MULT)
            acc = ps_m.tile([128, 128], F32, tag="macc")
            nc.tensor.matmul(
                acc, identf[:], h[:, t, c0 : c0 + 128], start=True, stop=False
            )
            for r in range(16):
                mv = _bview(prod[:], r, [[256, 8], [16, 16]])
                nc.tensor.matmul(acc, ident[:], mv, start=False, stop=(r == 15))
            with nc.allow_low_precision(reason="h is fp32r"):
                if cp_eng == "act":
                    nc.scalar.copy(h[:, t, c0 : c0 + 128], acc)
                else:
                    nc.vector.tensor_copy(out=h[:, t, c0 : c0 + 128], in_=acc)

        def emit_global_vt(t, hf, rb, vt):
            """Global-mixer normalized input, written within-chunk
            transposed (v_t[c, 16j+i]) on Pool."""
            c0 = 128 * hf
            i0 = 8 * hf
            nc.vector.tensor_tensor(
                out=_bview(vt[:], i0, [[1, 8], [16, 16]]),
                in0=_bview(h[:, t, :], c0, [[16, 8], [1, 16]]),
                in1=_bview(rb[:], 0, [[16, 8], [1, 16]]),
                op=MULT,
            )

        def emit_global_tile(t, vt, wm_sb, cp_eng):
            """Global-mixer tile: full 2x product over all i, 16 identity
            matmuls + h-inclusion in one PSUM group, copy-back."""
            prod = ppool.tile([128, 16, 16, 16], BF16, tag="gprod")
            vv = _bview(vt[:], 0, [[0, 16], [16, 16], [1, 16]])
            wv_ = _bview(wm_sb[:, t, :], 0, [[16, 16], [0, 16], [1, 16]])
            nc.vector.tensor_tensor(out=prod, in0=vv, in1=wv_, op=MULT)
            acc = ps_m.tile([128, S], F32, tag="macc", name="gacc")
            nc.tensor.matmul(acc, identf[:], h[:, t, :], start=True, stop=False)
            for r in range(16):
                mv = _bview(prod[:], r, [[256, 16], [16, 16]])
                nc.tensor.matmul(acc, ident[:], mv, start=False, stop=(r == 15))
            with nc.allow_low_precision(reason="h is fp32r"):
                if cp_eng == "act":
                    nc.scalar.copy(h[:, t, :], acc)
                    nc.vector.tensor_copy(out=hb[:, t, :], in_=acc)
                else:
                    nc.vector.tensor_copy(out=h[:, t, :], in_=acc)
                    nc.scalar.copy(hb[:, t, :], acc)

        for l in range(n_layers):
            wv_sb = wpool.tile([128, NT, GLU], BF16, tag="wv")
            nc.sync.dma_start(
                out=wv_sb, in_=wv_d[l].rearrange("(t p) o -> p t o", p=128)
            )
            wg_sb = wpool.tile([128, NT, GLU], BF16, tag="wg")
            nc.sync.dma_start(
                out=wg_sb, in_=wg_d[l].rearrange("(t p) o -> p t o", p=128)
            )
            wo_sb = wpool.tile([128, GT, H], BF16, tag="wo")
            nc.sync.dma_start(
                out=wo_sb, in_=wo_d[l].rearrange("(t p) c -> p t c", p=128)
            )
            wl_sb = wpool.tile([128, NT, 256], BF16, tag="wl")
            nc.sync.dma_start(
                out=wl_sb, in_=wl_d[l].rearrange("(t p) q -> p t q", p=128)
            )
            wm_sb = wpool.tile([128, NT, 256], BF16, tag="wm")
            nc.sync.dma_start(
                out=wm_sb, in_=wm_d[l].rearrange("(t p) q -> p t q", p=128)
            )
            nc.tensor.ldweights(wv_sb[:, 0, 0:128])
            nc.tensor.ldweights(wg_sb[:, 0, 0:128])
            nc.tensor.ldweights(wo_sb[:, 0, 0:128])

            # ---------- local mixer: out[c,i,p] = sum_j Wl[c,p,j] u[c,i,j]
            # A/B column halves pipelined; each following rms chain is
            # emitted two tiles into the preceding product stream.
            us = [apool.tile([128, S], BF16, tag=f"u{t}", name=f"u{t}") for t in range(NT)]
            vts = [apool.tile([128, S], BF16, tag=f"v{t}", name=f"v{t}") for t in range(NT)]
            cps = ["dve", "act", "dve", "act"]
            rb1a = emit_rms(0, 128)
            emit_local_tile(0, 0, rb1a, us[0], wl_sb, cps[0])
            emit_local_tile(1, 0, rb1a, us[1], wl_sb, cps[1])
            rb1b = emit_rms(128, 128)
            emit_local_tile(2, 0, rb1a, us[2], wl_sb, cps[2])
            emit_local_tile(3, 0, rb1a, us[3], wl_sb, cps[3])
            emit_local_tile(0, 1, rb1b, us[0], wl_sb, cps[0])
            emit_local_tile(1, 1, rb1b, us[1], wl_sb, cps[1])
            rb2a = emit_rms(0, 128)
            emit_local_tile(2, 1, rb1b, us[2], wl_sb, cps[2])
            emit_local_tile(3, 1, rb1b, us[3], wl_sb, cps[3])

            # ---------- global mixer: out[c,p,j] = sum_i Wg[c,p,i] v[c,i,j]
            # (vts allocated at the top of the local mixer so vtA mults can
            # be emitted into the local-B stream)
            for t in range(NT):
                emit_global_vt(t, 0, rb2a, vts[t])
            rb2b = emit_rms(128, 128)
            for t in range(NT):
                emit_global_vt(t, 1, rb2b, vts[t])
            emit_global_tile(0, vts[0], wm_sb, "dve")
            emit_global_tile(1, vts[1], wm_sb, "act")
            emit_global_tile(2, vts[2], wm_sb, "dve")
            emit_global_tile(3, vts[3], wm_sb, "act")

            # ---------- GLU MLP (rstd factored out of the channel sums)
            # p1 = Wv@h, p3 = Wg@h run on raw h, overlapping the rms chain;
            # h1 = p1*rstd, h3 = p3*rstd, h += Wo @ (silu(h1) . h3) + h via
            # PSUM-inclusion.
            gts = []
            rb3 = None
            for ot in range(GT):
                p1 = ps_g.tile([128, S], F32, tag="pg")
                for kt in range(NT):
                    nc.tensor.matmul(
                        p1,
                        wv_sb[:, kt, ot * 128 : (ot + 1) * 128],
                        hb[:, kt, :],
                        start=(kt == 0),
                        stop=(kt == NT - 1),
                    )
                if ot == 0:
                    # rms3 chain overlaps the p1/p3 matmul stream
                    rb3 = emit_rms(0, 256)
                    # warm the silu table while the matmuls run
                    nc.scalar.activation(dum_b, dum_a, silu_f)
                h1 = apool.tile([128, S], BF16, tag="h1")
                nc.vector.tensor_tensor(out=h1, in0=p1, in1=rb3, op=MULT)
                s1 = apool.tile([128, S], BF16, tag="s1")
                if sim_compat:
                    # CoreSim has no Silu: emulate with Sigmoid + extra mul
                    sg = apool.tile([128, S], BF16, tag="sg")
                    nc.scalar.activation(sg, h1, AF.Sigmoid)
                    nc.vector.tensor_tensor(out=s1, in0=sg, in1=h1, op=MULT)
                else:
                    nc.scalar.activation(s1, h1, AF.Silu)
                p3 = ps_g.tile([128, S], F32, tag="pg")
                for kt in range(NT):
                    nc.tensor.matmul(
                        p3,
                        wg_sb[:, kt, ot * 128 : (ot + 1) * 128],
                        hb[:, kt, :],
                        start=(kt == 0),
                        stop=(kt == NT - 1),
                    )
                h3 = apool.tile([128, S], BF16, tag="h3")
                nc.vector.tensor_tensor(out=h3, in0=p3, in1=rb3, op=MULT)
                gt_ = gpool.tile([128, S], BF16, tag=f"g{ot}", name=f"g{ot}")
                nc.gpsimd.tensor_tensor(out=gt_, in0=s1, in1=h3, op=MULT)
                gts.append(gt_)
            # warm the sqrt table for the next layer's rms
            nc.scalar.activation(dum_b, dum_a, AF.Sqrt)
            for t in range(NT):
                po = ps_o.tile([128, S], F32, tag="po")
                nc.tensor.matmul(po, identf[:], h[:, t, :], start=True, stop=False)
                for ot in range(GT):
                    nc.tensor.matmul(
                        po,
                        wo_sb[:, ot, t * 128 : (t + 1) * 128],
                        gts[ot][:],
                        start=False,
                        stop=(ot == GT - 1),
                    )
                with nc.allow_low_precision(reason="h is fp32r"):
                    nc.vector.tensor_copy(out=h[:, t, :], in_=po)

        # ---------- head ----------
        hw_sb = singles.tile([128, NT, VOC], BF16, tag="hw")
        nc.sync.dma_start(out=hw_sb, in_=hw_d.rearrange("(t p) v -> p t v", p=128))
        nc.tensor.ldweights(hw_sb[:, 0, 0:128])
        # head: logits = (head_w*head_rms*scale) @ (h * rstd); rstd factored
        # out of the channel sum as well.
        rbh = emit_rms(0, 256)
        with nc.allow_low_precision(reason="bf16 head input"):
            for t in range(NT):
                nc.vector.tensor_copy(out=hb[:, t, :], in_=h[:, t, :])
        for mc in range(VOC // 128):
            po = ps_o.tile([128, S], F32, tag="po")
            for kt in range(NT):
                nc.tensor.matmul(
                    po,
                    hw_sb[:, kt, mc * 128 : (mc + 1) * 128],
                    hb[:, kt, :],
                    start=(kt == 0),
                    stop=(kt == NT - 1),
                )
            ot_sb = apool.tile([128, S], F32, tag="osb")
            nc.vector.tensor_tensor(out=ot_sb, in0=po, in1=rbh, op=MULT)
            nc.sync.dma_start(out=out_d[mc * 128 : (mc + 1) * 128, :], in_=ot_sb)

    nc.compile()
    return nc


def _prep_inputs(inputs, n_layers=L):
    """Host-side weight folding + layout prep. Returns dict of np arrays."""
    f = lambda k: np.asarray(inputs[k], dtype=np.float32)
    x = f("x")
    stem_w = f("stem_w")  # [H, CIN]
    rl, rg, rf = f("rms_local"), f("rms_global"), f("rms_ffn")
    al, ag, am = f("alpha_local"), f("alpha_global"), f("alpha_mlp")
    w_local, w_global = f("w_local"), f("w_global")  # [L, H, BLK, BLK]
    wv, wg, wo = f("wv"), f("wg"), f("wo")
    head_rms, head_w = f("head_rms"), f("head_w")
    hls = np.float32(np.asarray(inputs["head_logit_scale"]))

    bf = ml_dtypes.bfloat16
    nl = n_layers

    # local: fold alpha_local * rms_local[c] into Wl[c,p,j]; layout [c, 16p+j]
    wl_h = (w_local[:nl] * al[:nl, None, None, None] * rl[:nl, :, None, None]).reshape(
        nl, H, 256
    )
    # global: Wg[c,p,i]; layout [c, 16p+i]
    wm_h = (w_global[:nl] * ag[:nl, None, None, None] * rg[:nl, :, None, None]).reshape(
        nl, H, 256
    )
    # GLU: fold rms_ffn into wv/wg columns; alpha_mlp into wo
    wvT = np.ascontiguousarray(
        np.transpose(wv[:nl] * rf[:nl, None, :], (0, 2, 1))
    )  # [L, H, GLU]
    wgT = np.ascontiguousarray(np.transpose(wg[:nl] * rf[:nl, None, :], (0, 2, 1)))
    woT = np.ascontiguousarray(
        np.transpose(wo[:nl] * am[:nl, None, None], (0, 2, 1))
    )  # [L, GLU, H]
    headT = np.ascontiguousarray((head_w * head_rms[None, :] * hls).T)  # [H, VOC]

    stw_pad = np.zeros((384, H), np.float32)
    stw_pad[:CIN] = stem_w.T
    common = {
        "stem_wT": stw_pad,  # [384, H] zero-padded
        "wvT": wvT.astype(bf),
        "wgT": wgT.astype(bf),
        "woT": woT.astype(bf),
        "wl": wl_h.astype(bf),
        "wm": wm_h.astype(bf),
        "headT": headT.astype(bf),
        "ident": np.eye(128, dtype=bf),
        "identf": np.eye(128, dtype=np.float32),
        "ones_k": np.ones((128, 1), bf),
        "ones_m": np.ones((1, 128), bf),
    }
    per_core = []
    for b in range(B):
        xp = np.zeros((384, S), np.float32)
        xp[:CIN] = x[b, :, 0, :]
        per_core.append(dict(common, x=xp))
    return per_core


def run(inputs, n_layers=L, trace=False):
    key = n_layers
    if key not in _PROG_CACHE:
        _PROG_CACHE[key] = build_program(n_layers)
    nc = _PROG_CACHE[key]
    in_maps = _prep_inputs(inputs, n_layers)
    res = run_bass_kernel_spmd(nc, in_maps, core_ids=list(range(B)), trace=trace)
    out = np.stack([r["out"] for r in res.results])  # [B, VOC, S]
    return out[:, :, None, :].astype(np.float32), res


def kernel(**inputs):
    out, _ = run(inputs, L, trace=False)
    return out

